# revision 24
# baseline (speedup 1.0000x reference)
"""MoE layer (E=8 experts, top-2 routing) on 8 Trainium2 NeuronCores.

Strategy: load-balanced expert-parallel. The host computes the (tiny) gating
network in fp64 -- logits = x @ wg + bg, top-2, softmax -- and dispatches each
token to the cores owning its two selected experts. Core c runs expert c's FFN
    y = relu(x_e @ w1[e] + b1[e]) @ w2[e]
over segment A (first NT1 tokens routed to expert c) and, to flatten the
expert-count imbalance, a short segment B holding overflow tokens of some
donor expert (whose weights are loaded as a second weight set). Rows are
scaled by the gate weight on-device; the host scatter-adds the two slots per
token back together (plus the combine@b2 bias term).

Device-side schedule notes:
  - weights stream on the scalar-engine HWDGE queue, x/gate/y on the sync
    queue, so the startup crunch (w1+w2+x0+x1 before the pipeline fills) runs
    on two descriptor generators instead of one.
  - mm1 of chunk c+1 is issued before mm2 of chunk c so the w2 arrival time
    stays off the critical path.
  - y is written back fp16 (half the store traffic of fp32; quantization is
    ~1e-4 relative, far below the bf16 matmul error).

All device inputs are host-permuted so that every SBUF partition's data is
one contiguous DRAM run (fat DMA lines -> high per-engine DMA rate).

Hardcoded problem shape: x [4,4096,512], w1 [8,512,1024], w2 [8,1024,512],
wg [512,8], top_k=2.
"""

import os
import numpy as np

B, S, D, F, E = 4, 4096, 512, 1024, 8
TOP_K = 2
N_CORES = 8
KD = D // 128   # contraction blocks for mm1
FB = F // 128   # F blocks (h partition blocks / mm2 contraction blocks)
NT1 = 3968     # segment A length (own expert), multiple of 128
NT2 = 256      # segment B length (overflow tokens, donor expert weights)

TRACE = os.environ.get("MOE_TRACE", "0") == "1"

_PROGRAM_CACHE = {}


def _chunk_plan(nt2):
    """Chunk sizes + DMA groups for NT = NT1 + nt2 tokens.

    All chunks are >=256 tokens: matmuls shorter than ~384 moving rows are
    LDWEIGHTS-cadence-bound (~213ns each regardless of rows), so small
    chunks run at a fraction of peak. Group 0 is a single chunk (the first
    transfer gates the pipeline start); later groups pair chunks for fat
    DMA lines. Segment-B chunks get their own group(s).
    """
    assert NT1 % 128 == 0 and NT1 >= 1536
    # 256-token lead chunks: the pipeline starts on the first 0.25MB of x.
    # (256 moving rows ~ LDWEIGHTS time, so 256 is the smallest chunk that
    # still streams at full rate; 128-row chunks would be LDW-bound.)
    rest = NT1 - 512
    chunks_a = [256, 256] + [512] * (rest // 512)
    if rest % 512:
        chunks_a.append(rest % 512)
    chunks_b = [512] * (nt2 // 512)
    if nt2 % 512:
        chunks_b.append(nt2 % 512)
    chunks = chunks_a + chunks_b
    na = len(chunks_a)
    groups = [[0], [1], [2]]
    i = 3
    while i < na:
        groups.append(list(range(i, min(i + 2, na))))
        i += 2
    i = na
    while i < len(chunks):
        groups.append(list(range(i, min(i + 2, len(chunks)))))
        i += 2
    return chunks, groups, na


def _build_program(nt2):
    from concourse import bacc, tile, mybir

    dt = mybir.dt
    DT = dt.bfloat16

    nc = bacc.Bacc("TRN2", target_bir_lowering=False, debug=False)

    NT = NT1 + nt2
    NTG = NT // 128
    chunks, groups, na = _chunk_plan(nt2)
    offs = [sum(chunks[:i]) for i in range(len(chunks) + 1)]

    # host-permuted inputs: per-partition contiguous runs
    xp_d = nc.dram_tensor("xp", [128, KD * NT], DT, kind="ExternalInput").ap()
    w1a_d = nc.dram_tensor("w1a", [128, FB * KD * 128], DT, kind="ExternalInput").ap()
    w1b_d = nc.dram_tensor("w1b", [128, FB * KD * 128], DT, kind="ExternalInput").ap()
    w2a_d = nc.dram_tensor("w2a", [128, FB * D], DT, kind="ExternalInput").ap()
    w2b_d = nc.dram_tensor("w2b", [128, FB * D], DT, kind="ExternalInput").ap()
    # b1A | b1B | gate packed into one small transfer (each DMA transfer
    # costs ~0.7us of trigger + queue latency, so merge the tiny ones)
    aux_d = nc.dram_tensor("aux", [128, 2 * FB + NTG], dt.float32,
                           kind="ExternalInput").ap()
    y_d = nc.dram_tensor("y", [NT, D], dt.float16, kind="ExternalOutput").ap()

    with tile.TileContext(nc) as tc:
        with (
            tc.tile_pool(name="w", bufs=1) as wpool,
            tc.tile_pool(name="x", bufs=3) as xpool,
            tc.tile_pool(name="h", bufs=3) as hpool,
            tc.tile_pool(name="o", bufs=4) as opool,
            tc.tile_pool(name="ps1", bufs=4, space="PSUM") as ps1,
            tc.tile_pool(name="ps2", bufs=4, space="PSUM") as ps2,
        ):
            # All loads go on the sync-engine HWDGE queue in strict need
            # order (queue FIFO order == delivery priority; the global DMA
            # rate is HBM-share-capped, so order is what matters). y stores
            # ride the scalar-engine queue so a deep load FIFO never delays
            # them. One trigger per transfer: packets of a single transfer
            # already round-robin across all 16 DMA engines, and each
            # trigger costs ~0.7us of issuing-engine time.
            x_tiles = {}

            def load_group(g):
                cidx = groups[g]
                goff = offs[cidx[0]]
                gs = sum(chunks[c] for c in cidx)
                x_sb = xpool.tile([128, KD, gs], DT, tag="x")
                nc.sync.dma_start(
                    out=x_sb[:], in_=xp_d[:, KD * goff:KD * (goff + gs)])
                x_tiles[g] = x_sb

            # Startup schedule (measured: the first transfers of the two
            # HWDGE queues serialize against each other and Q10 adds ~2.4us
            # trigger-to-data latency, so the critical chain -- x chunk 0 +
            # w1A fb-blocks in exact PE consumption order -- lives on ONE
            # queue; loose-deadline loads go to the other):
            #   sync:   xg0, w1a fb0-1, fb2-3, fb4-7, xg1, xg2, [prefetches]
            #   scalar: w2a halves, y stores, B weights
            #   gpsimd: aux (tiny 196B-line transfer -- 128 small packets
            #           would clog an HWDGE queue for ~2us mid-crunch)
            load_group(0)

            w1a_sb = wpool.tile([128, FB, KD, 128], DT)
            QW = FB * KD * 128 // 4
            nc.sync.dma_start(out=w1a_sb[:, :2], in_=w1a_d[:, :QW])
            nc.sync.dma_start(out=w1a_sb[:, 2:4], in_=w1a_d[:, QW:2 * QW])
            nc.sync.dma_start(out=w1a_sb[:, 4:], in_=w1a_d[:, 2 * QW:])

            aux_sb = wpool.tile([128, 2 * FB + NTG], dt.float32)
            nc.gpsimd.dma_start(out=aux_sb[:], in_=aux_d[:])
            b1a_sb = aux_sb[:, 0:FB]
            b1b_sb = aux_sb[:, FB:2 * FB]
            g_sb = aux_sb[:, 2 * FB:]

            load_group(1)
            load_group(2)

            w2a_sb = wpool.tile([128, FB, D], DT)
            WH = FB * D // 2
            nc.scalar.dma_start(out=w2a_sb[:, :FB // 2], in_=w2a_d[:, :WH])
            nc.scalar.dma_start(out=w2a_sb[:, FB // 2:], in_=w2a_d[:, WH:])

            # B-set (overflow segment) weights: issued later on the scalar
            # queue; they are only needed by the final short chunk.
            w1b_sb = wpool.tile([128, FB, KD, 128], DT)
            w2b_sb = wpool.tile([128, FB, D], DT)

            def load_b_weights():
                nc.scalar.dma_start(out=w1b_sb[:], in_=w1b_d[:])
                nc.scalar.dma_start(out=w2b_sb[:], in_=w2b_d[:])

            chunk_group = {}
            for g, cidx in enumerate(groups):
                for c in cidx:
                    chunk_group[c] = g

            def mm1(c):
                g = chunk_group[c]
                # prefetch the next x group when the first chunk of the
                # current group starts (groups 0 and 1 are pre-issued)
                if c == groups[g][0]:
                    if g == 3:
                        load_b_weights()
                    if g + 1 < len(groups) and g + 1 >= 3:
                        load_group(g + 1)
                x_sb = x_tiles[g]
                goff = offs[groups[g][0]]
                cs = chunks[c]
                lo = offs[c] - goff
                w1_sb = w1a_sb if c < na else w1b_sb
                b1_sb = b1a_sb if c < na else b1b_sb
                h_sb = hpool.tile([128, FB, cs], DT, tag="h")
                for fb in range(FB):
                    p = ps1.tile([128, cs], mybir.dt.float32, tag="ps1")
                    for kc in range(KD):
                        nc.tensor.matmul(
                            p[:],
                            w1_sb[:, fb, kc, :],
                            x_sb[:, kc, lo:lo + cs],
                            start=(kc == 0),
                            stop=(kc == KD - 1),
                        )
                    nc.scalar.activation(
                        h_sb[:, fb, :],
                        p[:],
                        mybir.ActivationFunctionType.Relu,
                        bias=b1_sb[:, fb:fb + 1],
                        scale=1.0,
                    )
                return h_sb

            def mm2(c, h_sb):
                cs = chunks[c]
                off = offs[c]
                w2_sb = w2a_sb if c < na else w2b_sb
                last_c = c == len(chunks) - 1
                for tb in range(cs // 128):
                    # split the very last tb into two half-D pieces so the
                    # final vector op + y store overlap the final matmuls
                    # instead of serializing after them
                    last_tb = last_c and tb == cs // 128 - 1
                    for dh in range(2 if last_tb else 1):
                        dw = 256 if last_tb else 512
                        p2 = ps2.tile([128, dw], mybir.dt.float32, tag="ps2")
                        for fb in range(FB):
                            nc.tensor.matmul(
                                p2[:],
                                h_sb[:, fb, tb * 128:(tb + 1) * 128],
                                w2_sb[:, fb, dh * dw:dh * dw + dw],
                                start=(fb == 0),
                                stop=(fb == FB - 1),
                            )
                        o_sb = opool.tile([128, dw], mybir.dt.float16, tag="o")
                        nc.vector.tensor_scalar_mul(
                            o_sb[:], p2[:],
                            g_sb[:, off // 128 + tb:off // 128 + tb + 1]
                        )
                        nc.scalar.dma_start(
                            out=y_d[off + tb * 128:off + (tb + 1) * 128,
                                    dh * dw:dh * dw + dw],
                            in_=o_sb[:],
                        )

            # skewed pipeline: mm1(c+1) is emitted before mm2(c) so mm2's
            # weight (w2) arrival never blocks the tensor engine early on
            h_prev = mm1(0)
            for c in range(1, len(chunks)):
                h_cur = mm1(c)
                mm2(c - 1, h_prev)
                h_prev = h_cur
            mm2(len(chunks) - 1, h_prev)
    nc.compile()
    return nc


def _install_ntff_hook():
    """Register the axon NTFF profiling hook that run_bass_kernel_spmd
    (trace=True) looks for under antenv.axon_hooks; this container's antenv
    lacks that module, so recreate it via ctypes against libaxon_pjrt.so."""
    import sys, types, ctypes, contextlib

    if "antenv.axon_hooks" in sys.modules:
        return
    try:
        lib = ctypes.CDLL("/opt/axon/libaxon_pjrt.so")
    except OSError:
        return
    if not hasattr(lib, "axon_start_nrt_profile"):
        return
    lib.axon_start_nrt_profile.argtypes = [ctypes.POINTER(ctypes.c_int64), ctypes.c_size_t]
    lib.axon_start_nrt_profile.restype = ctypes.c_int64
    lib.axon_stop_nrt_profile.argtypes = [ctypes.c_char_p]
    lib.axon_stop_nrt_profile.restype = ctypes.c_int64

    @contextlib.contextmanager
    def _hook(output_dir, device_ids):
        import jax

        jax.devices()
        if device_ids:
            ids = (ctypes.c_int64 * len(device_ids))(*device_ids)
            rc = lib.axon_start_nrt_profile(ids, len(device_ids))
        else:
            rc = lib.axon_start_nrt_profile(None, 0)
        if rc != 0:
            raise RuntimeError(f"axon_start_nrt_profile rc={rc}")
        try:
            yield
        finally:
            n = lib.axon_stop_nrt_profile(str(output_dir).encode())
            print(f"profile: {n} ntff file(s) written to {output_dir}")

    mod = types.ModuleType("antenv.axon_hooks")
    _holder = {"h": _hook}
    mod.set_axon_ntff_profile_hook = lambda h: _holder.__setitem__("h", h)
    mod.get_axon_ntff_profile_hook = lambda: _holder["h"]
    sys.modules["antenv.axon_hooks"] = mod

    # avoid the S3/Fish artifact upload in the trace post-processing path
    import concourse.bass_utils as bu

    bu.upload_artifacts = lambda tmpdir: str(tmpdir)


def kernel(**inputs):
    import ml_dtypes
    from concourse.bass_utils import run_bass_kernel_spmd

    if TRACE:
        _install_ntff_hook()

    x = np.asarray(inputs["x"], np.float32)
    w1 = np.asarray(inputs["w1"], np.float32)
    b1 = np.asarray(inputs["b1"], np.float32)
    w2 = np.asarray(inputs["w2"], np.float32)
    b2 = np.asarray(inputs["b2"], np.float32)
    wg = np.asarray(inputs["wg"], np.float32)
    bg = np.asarray(inputs["bg"], np.float32)

    T = x.shape[0] * x.shape[1]
    xf = x.reshape(T, D)

    # ---- host gating (fp64): logits -> top-2 (jax.lax.top_k tie order:
    # lower index wins -> stable argsort on -logits) -> softmax over top-2.
    logits = xf.astype(np.float64) @ wg.astype(np.float64) + bg.astype(np.float64)
    order = np.argsort(-logits, axis=1, kind="stable")
    top_idx = order[:, :TOP_K]                      # [T, K]
    top_vals = np.take_along_axis(logits, top_idx, axis=1)
    gwts = np.exp(top_vals - top_vals.max(axis=1, keepdims=True))
    gwts = gwts / gwts.sum(axis=1, keepdims=True)   # [T, K]

    # ---- dispatch: sort slots (t, k) by expert; per-expert contiguous runs.
    flat_expert = top_idx.ravel()                   # slot s = t*K + k
    perm = np.argsort(flat_expert, kind="stable")   # slots grouped by expert
    counts = np.bincount(flat_expert, minlength=E)
    cum = np.concatenate([[0], np.cumsum(counts)])
    slot_tok = perm // TOP_K                        # token of each sorted slot
    gates_sorted = gwts.ravel()[perm].astype(np.float32)

    # ---- load balance: core e owns expert e's first <=NT1 slots (segment A);
    # overflow beyond NT1 is cut into NT2-sized pieces placed in other cores'
    # segment B (with the donor expert's weights as the second weight set).
    nt2 = NT2
    while True:
        pieces = []
        for e in range(E):
            for s0 in range(NT1, int(counts[e]), nt2):
                pieces.append((e, cum[e] + s0, min(nt2, int(counts[e]) - s0)))
        if len(pieces) <= N_CORES:
            break
        nt2 += 128
    NT = NT1 + nt2
    NTG = NT // 128
    chunks, groups, _na = _chunk_plan(nt2)
    offs = [sum(chunks[:i]) for i in range(len(chunks) + 1)]
    group_bounds = [(offs[ci[0]], offs[ci[-1] + 1]) for ci in groups]

    io_dtype = ml_dtypes.bfloat16

    def permute_x(xt):
        # xt [D, NT] -> [128, sum_g KD*gs]: per partition, per group,
        # (kc, token) contiguous
        xr = xt.reshape(KD, 128, NT)
        parts = [
            xr[:, :, g0:g1].transpose(1, 0, 2).reshape(128, -1)
            for (g0, g1) in group_bounds
        ]
        return np.ascontiguousarray(np.concatenate(parts, axis=1))

    def w1p(e):
        # [D, F] -> [128, FB, KD, 128] flattened: fb-major weight blocks
        return np.ascontiguousarray(
            w1[e].astype(io_dtype).reshape(KD, 128, FB, 128)
            .transpose(1, 2, 0, 3).reshape(128, FB * KD * 128))

    def w2p(e):
        return np.ascontiguousarray(
            w2[e].astype(io_dtype).reshape(FB, 128, D)
            .transpose(1, 0, 2).reshape(128, FB * D))

    def b1c(e):
        return np.ascontiguousarray(b1[e].reshape(FB, 128).T)

    in_maps = []
    core_sel = []   # (sorted-slot indices, device rows) per core
    for c in range(E):
        nA = min(int(counts[c]), NT1)
        slots_a = np.arange(cum[c], cum[c] + nA)
        if c < len(pieces):
            eB, sB0, nB = pieces[c]
            slots_b = np.arange(sB0, sB0 + nB)
        else:
            eB, slots_b = c, np.arange(0)
        sel = np.concatenate([slots_a, slots_b])
        rows = np.concatenate([np.arange(nA), NT1 + np.arange(len(slots_b))])
        core_sel.append((sel, rows))

        xt = np.zeros((D, NT), io_dtype)
        xt[:, rows] = xf[slot_tok[sel]].astype(io_dtype).T
        gate = np.zeros(NT, np.float32)
        gate[rows] = gates_sorted[sel]
        in_maps.append({
            "xp": permute_x(xt),
            "w1a": w1p(c), "w1b": w1p(eB),
            "w2a": w2p(c), "w2b": w2p(eB),
            "aux": np.ascontiguousarray(np.concatenate(
                [b1c(c), b1c(eB), gate.reshape(NTG, 128).T], axis=1)),
        })

    def run_device():
        if nt2 not in _PROGRAM_CACHE:
            _PROGRAM_CACHE[nt2] = _build_program(nt2)
        nc = _PROGRAM_CACHE[nt2]
        res = run_bass_kernel_spmd(nc, in_maps, list(range(N_CORES)), trace=TRACE)
        if TRACE and res.exec_time_ns is not None:
            print(f"HW exec time: {res.exec_time_ns} ns")
        return [res.results[c]["y"] for c in range(N_CORES)]

    out_slots = np.zeros((T * TOP_K, D), np.float32)
    try:
        try:
            y_cores = run_device()
        except Exception:
            # transient device errors (e.g. NRT exec-unit unrecoverable)
            # are usually gone on retry with a freshly built program
            _PROGRAM_CACHE.clear()
            y_cores = run_device()
        for c in range(N_CORES):
            sel, rows = core_sel[c]
            out_slots[perm[sel]] = y_cores[c].astype(np.float32)[rows]
    except Exception as exc:
        # last resort: identical math on the host so the result is still
        # correct even if the accelerator path is down
        import sys
        print(f"device path failed twice ({exc!r}); computing FFN on host",
              file=sys.stderr)
        for e in range(E):
            n = int(counts[e])
            toks = slot_tok[cum[e]:cum[e] + n]
            h = np.maximum(xf[toks] @ w1[e] + b1[e], 0.0)
            y = (h @ w2[e]) * gates_sorted[cum[e]:cum[e] + n, None]
            out_slots[perm[cum[e]:cum[e] + n]] = y.astype(np.float32)

    # ---- unshard: sum the K slots per token, add the combine@b2 bias term.
    out = out_slots.reshape(T, TOP_K, D).sum(axis=1)
    combine = np.zeros((T, E), np.float32)
    np.put_along_axis(combine, top_idx, gwts.astype(np.float32), axis=1)
    out += combine @ b2

    return out.reshape(B, S, D).astype(np.float32)


# revision 28
# speedup vs baseline: 1.0268x; 1.0268x over previous
"""MoE layer (E=8 experts, top-2 routing) on 8 Trainium2 NeuronCores.

Strategy: load-balanced expert-parallel. The host computes the (tiny) gating
network in fp64 -- logits = x @ wg + bg, top-2, softmax -- and dispatches each
token to the cores owning its two selected experts. Core c runs expert c's FFN
    y = relu(x_e @ w1[e] + b1[e]) @ w2[e]
over segment A (first NT1 tokens routed to expert c) and, to flatten the
expert-count imbalance, a short segment B holding overflow tokens of some
donor expert (whose weights are loaded as a second weight set). Rows are
scaled by the gate weight on-device; the host scatter-adds the two slots per
token back together (plus the combine@b2 bias term).

Device-side schedule notes:
  - weights stream on the scalar-engine HWDGE queue, x/gate/y on the sync
    queue, so the startup crunch (w1+w2+x0+x1 before the pipeline fills) runs
    on two descriptor generators instead of one.
  - mm1 of chunk c+1 is issued before mm2 of chunk c so the w2 arrival time
    stays off the critical path.
  - y is written back fp16 (half the store traffic of fp32; quantization is
    ~1e-4 relative, far below the bf16 matmul error).

All device inputs are host-permuted so that every SBUF partition's data is
one contiguous DRAM run (fat DMA lines -> high per-engine DMA rate).

Hardcoded problem shape: x [4,4096,512], w1 [8,512,1024], w2 [8,1024,512],
wg [512,8], top_k=2.
"""

import os
import numpy as np

B, S, D, F, E = 4, 4096, 512, 1024, 8
TOP_K = 2
N_CORES = 8
KD = D // 128   # contraction blocks for mm1
FB = F // 128   # F blocks (h partition blocks / mm2 contraction blocks)
NT1 = 3968     # segment A length (own expert), multiple of 128
NT2 = 256      # segment B length (overflow tokens, donor expert weights)

TRACE = os.environ.get("MOE_TRACE", "0") == "1"

_PROGRAM_CACHE = {}


def _chunk_plan(nt2):
    """Chunk sizes + DMA groups for NT = NT1 + nt2 tokens.

    All chunks are >=256 tokens: matmuls shorter than ~384 moving rows are
    LDWEIGHTS-cadence-bound (~213ns each regardless of rows), so small
    chunks run at a fraction of peak. Group 0 is a single chunk (the first
    transfer gates the pipeline start); later groups pair chunks for fat
    DMA lines. Segment-B chunks get their own group(s).
    """
    assert NT1 % 128 == 0 and NT1 >= 1024
    chunks_a = [512] * (NT1 // 512)
    if NT1 % 512:
        chunks_a.append(NT1 % 512)
    chunks_b = [512] * (nt2 // 512)
    if nt2 % 512:
        chunks_b.append(nt2 % 512)
    chunks = chunks_a + chunks_b
    na = len(chunks_a)
    groups = [[0]]
    i = 1
    while i < na:
        groups.append(list(range(i, min(i + 2, na))))
        i += 2
    i = na
    while i < len(chunks):
        groups.append(list(range(i, min(i + 2, len(chunks)))))
        i += 2
    return chunks, groups, na


def _build_program(nt2):
    from concourse import bacc, tile, mybir

    dt = mybir.dt
    DT = dt.bfloat16

    nc = bacc.Bacc("TRN2", target_bir_lowering=False, debug=False)

    NT = NT1 + nt2
    NTG = NT // 128
    chunks, groups, na = _chunk_plan(nt2)
    offs = [sum(chunks[:i]) for i in range(len(chunks) + 1)]

    # host-permuted inputs: per-partition contiguous runs
    xp_d = nc.dram_tensor("xp", [128, KD * NT], DT, kind="ExternalInput").ap()
    w1a_d = nc.dram_tensor("w1a", [128, FB * KD * 128], DT, kind="ExternalInput").ap()
    w1b_d = nc.dram_tensor("w1b", [128, FB * KD * 128], DT, kind="ExternalInput").ap()
    w2a_d = nc.dram_tensor("w2a", [128, FB * D], DT, kind="ExternalInput").ap()
    w2b_d = nc.dram_tensor("w2b", [128, FB * D], DT, kind="ExternalInput").ap()
    # b1A | b1B | gate packed into one small transfer (each DMA transfer
    # costs ~0.7us of trigger + queue latency, so merge the tiny ones)
    aux_d = nc.dram_tensor("aux", [128, 2 * FB + NTG], dt.float32,
                           kind="ExternalInput").ap()
    y_d = nc.dram_tensor("y", [NT, D], dt.float16, kind="ExternalOutput").ap()

    with tile.TileContext(nc) as tc:
        with (
            tc.tile_pool(name="w", bufs=1) as wpool,
            tc.tile_pool(name="x", bufs=3) as xpool,
            tc.tile_pool(name="h", bufs=3) as hpool,
            tc.tile_pool(name="o", bufs=4) as opool,
            tc.tile_pool(name="ps1", bufs=4, space="PSUM") as ps1,
            tc.tile_pool(name="ps2", bufs=4, space="PSUM") as ps2,
        ):
            # All loads go on the sync-engine HWDGE queue in strict need
            # order (queue FIFO order == delivery priority; the global DMA
            # rate is HBM-share-capped, so order is what matters). y stores
            # ride the scalar-engine queue so a deep load FIFO never delays
            # them. One trigger per transfer: packets of a single transfer
            # already round-robin across all 16 DMA engines, and each
            # trigger costs ~0.7us of issuing-engine time.
            x_tiles = {}

            def load_group(g):
                cidx = groups[g]
                goff = offs[cidx[0]]
                gs = sum(chunks[c] for c in cidx)
                x_sb = xpool.tile([128, KD, gs], DT, tag="x")
                nc.sync.dma_start(
                    out=x_sb[:], in_=xp_d[:, KD * goff:KD * (goff + gs)])
                x_tiles[g] = x_sb

            # Startup schedule. Measured DMA behavior: rate warms from
            # ~120GB/s to ~350GB/s over the first ~8us, the two queues'
            # first transfers serialize against each other, and the scalar
            # queue adds ~2.4us trigger-to-data latency. So the critical
            # chain (xg0, w1a halves, xg1) owns the sync queue in exact PE
            # consumption order; aux soaks the scalar queue's initial
            # latency dead-zone; and w2a is triggered only after chunk 0's
            # activations, landing at warm DMA rate well before its first
            # use (mm2 of chunk 0 runs after mm1 of chunk 1 - the skew).
            load_group(0)

            w1a_sb = wpool.tile([128, FB, KD, 128], DT)
            HW = FB * KD * 128 // 2
            nc.sync.dma_start(out=w1a_sb[:, :FB // 2], in_=w1a_d[:, :HW])
            nc.sync.dma_start(out=w1a_sb[:, FB // 2:], in_=w1a_d[:, HW:])

            aux_sb = wpool.tile([128, 2 * FB + NTG], dt.float32)
            nc.scalar.dma_start(out=aux_sb[:], in_=aux_d[:])
            b1a_sb = aux_sb[:, 0:FB]
            b1b_sb = aux_sb[:, FB:2 * FB]
            g_sb = aux_sb[:, 2 * FB:]

            load_group(1)

            w2a_sb = wpool.tile([128, FB, D], DT)
            WH = FB * D // 2

            def load_w2a():
                nc.scalar.dma_start(out=w2a_sb[:, :FB // 2], in_=w2a_d[:, :WH])
                nc.scalar.dma_start(out=w2a_sb[:, FB // 2:], in_=w2a_d[:, WH:])

            # B-set (overflow segment) weights: issued later on the scalar
            # queue; they are only needed by the final short chunk.
            w1b_sb = wpool.tile([128, FB, KD, 128], DT)
            w2b_sb = wpool.tile([128, FB, D], DT)

            def load_b_weights():
                nc.scalar.dma_start(out=w1b_sb[:], in_=w1b_d[:])
                nc.scalar.dma_start(out=w2b_sb[:], in_=w2b_d[:])

            chunk_group = {}
            for g, cidx in enumerate(groups):
                for c in cidx:
                    chunk_group[c] = g

            def mm1(c):
                g = chunk_group[c]
                # prefetch the next x group when the first chunk of the
                # current group starts (groups 0 and 1 are pre-issued)
                if c == groups[g][0]:
                    if g == 2:
                        load_b_weights()
                    if g + 1 < len(groups) and g + 1 >= 2:
                        load_group(g + 1)
                x_sb = x_tiles[g]
                goff = offs[groups[g][0]]
                cs = chunks[c]
                lo = offs[c] - goff
                w1_sb = w1a_sb if c < na else w1b_sb
                b1_sb = b1a_sb if c < na else b1b_sb
                h_sb = hpool.tile([128, FB, cs], DT, tag="h")
                for fb in range(FB):
                    p = ps1.tile([128, cs], mybir.dt.float32, tag="ps1")
                    for kc in range(KD):
                        nc.tensor.matmul(
                            p[:],
                            w1_sb[:, fb, kc, :],
                            x_sb[:, kc, lo:lo + cs],
                            start=(kc == 0),
                            stop=(kc == KD - 1),
                        )
                    nc.scalar.activation(
                        h_sb[:, fb, :],
                        p[:],
                        mybir.ActivationFunctionType.Relu,
                        bias=b1_sb[:, fb:fb + 1],
                        scale=1.0,
                    )
                return h_sb

            def mm2(c, h_sb):
                cs = chunks[c]
                off = offs[c]
                w2_sb = w2a_sb if c < na else w2b_sb
                last_c = c == len(chunks) - 1
                for tb in range(cs // 128):
                    # split the very last tb into two half-D pieces so the
                    # final vector op + y store overlap the final matmuls
                    # instead of serializing after them
                    last_tb = last_c and tb == cs // 128 - 1
                    for dh in range(2 if last_tb else 1):
                        dw = 256 if last_tb else 512
                        p2 = ps2.tile([128, dw], mybir.dt.float32, tag="ps2")
                        for fb in range(FB):
                            nc.tensor.matmul(
                                p2[:],
                                h_sb[:, fb, tb * 128:(tb + 1) * 128],
                                w2_sb[:, fb, dh * dw:dh * dw + dw],
                                start=(fb == 0),
                                stop=(fb == FB - 1),
                            )
                        o_sb = opool.tile([128, dw], mybir.dt.float16, tag="o")
                        nc.vector.tensor_scalar_mul(
                            o_sb[:], p2[:],
                            g_sb[:, off // 128 + tb:off // 128 + tb + 1]
                        )
                        nc.scalar.dma_start(
                            out=y_d[off + tb * 128:off + (tb + 1) * 128,
                                    dh * dw:dh * dw + dw],
                            in_=o_sb[:],
                        )

            # skewed pipeline: mm1(c+1) is emitted before mm2(c) so mm2's
            # weight (w2) arrival never blocks the tensor engine early on
            h_prev = mm1(0)
            # w2a triggers sit on the scalar stream AFTER chunk 0's
            # activations: the transfer runs at warm DMA rate outside the
            # startup crunch window, still ~5us before mm2(c0) needs it
            load_w2a()
            for c in range(1, len(chunks)):
                h_cur = mm1(c)
                mm2(c - 1, h_prev)
                h_prev = h_cur
            mm2(len(chunks) - 1, h_prev)
    nc.compile()
    return nc


def _install_ntff_hook():
    """Register the axon NTFF profiling hook that run_bass_kernel_spmd
    (trace=True) looks for under antenv.axon_hooks; this container's antenv
    lacks that module, so recreate it via ctypes against libaxon_pjrt.so."""
    import sys, types, ctypes, contextlib

    if "antenv.axon_hooks" in sys.modules:
        return
    try:
        lib = ctypes.CDLL("/opt/axon/libaxon_pjrt.so")
    except OSError:
        return
    if not hasattr(lib, "axon_start_nrt_profile"):
        return
    lib.axon_start_nrt_profile.argtypes = [ctypes.POINTER(ctypes.c_int64), ctypes.c_size_t]
    lib.axon_start_nrt_profile.restype = ctypes.c_int64
    lib.axon_stop_nrt_profile.argtypes = [ctypes.c_char_p]
    lib.axon_stop_nrt_profile.restype = ctypes.c_int64

    @contextlib.contextmanager
    def _hook(output_dir, device_ids):
        import jax

        jax.devices()
        if device_ids:
            ids = (ctypes.c_int64 * len(device_ids))(*device_ids)
            rc = lib.axon_start_nrt_profile(ids, len(device_ids))
        else:
            rc = lib.axon_start_nrt_profile(None, 0)
        if rc != 0:
            raise RuntimeError(f"axon_start_nrt_profile rc={rc}")
        try:
            yield
        finally:
            n = lib.axon_stop_nrt_profile(str(output_dir).encode())
            print(f"profile: {n} ntff file(s) written to {output_dir}")

    mod = types.ModuleType("antenv.axon_hooks")
    _holder = {"h": _hook}
    mod.set_axon_ntff_profile_hook = lambda h: _holder.__setitem__("h", h)
    mod.get_axon_ntff_profile_hook = lambda: _holder["h"]
    sys.modules["antenv.axon_hooks"] = mod

    # avoid the S3/Fish artifact upload in the trace post-processing path
    import concourse.bass_utils as bu

    bu.upload_artifacts = lambda tmpdir: str(tmpdir)


def kernel(**inputs):
    import ml_dtypes
    from concourse.bass_utils import run_bass_kernel_spmd

    if TRACE:
        _install_ntff_hook()

    x = np.asarray(inputs["x"], np.float32)
    w1 = np.asarray(inputs["w1"], np.float32)
    b1 = np.asarray(inputs["b1"], np.float32)
    w2 = np.asarray(inputs["w2"], np.float32)
    b2 = np.asarray(inputs["b2"], np.float32)
    wg = np.asarray(inputs["wg"], np.float32)
    bg = np.asarray(inputs["bg"], np.float32)

    T = x.shape[0] * x.shape[1]
    xf = x.reshape(T, D)

    # ---- host gating (fp64): logits -> top-2 (jax.lax.top_k tie order:
    # lower index wins -> stable argsort on -logits) -> softmax over top-2.
    logits = xf.astype(np.float64) @ wg.astype(np.float64) + bg.astype(np.float64)
    order = np.argsort(-logits, axis=1, kind="stable")
    top_idx = order[:, :TOP_K]                      # [T, K]
    top_vals = np.take_along_axis(logits, top_idx, axis=1)
    gwts = np.exp(top_vals - top_vals.max(axis=1, keepdims=True))
    gwts = gwts / gwts.sum(axis=1, keepdims=True)   # [T, K]

    # ---- dispatch: sort slots (t, k) by expert; per-expert contiguous runs.
    flat_expert = top_idx.ravel()                   # slot s = t*K + k
    perm = np.argsort(flat_expert, kind="stable")   # slots grouped by expert
    counts = np.bincount(flat_expert, minlength=E)
    cum = np.concatenate([[0], np.cumsum(counts)])
    slot_tok = perm // TOP_K                        # token of each sorted slot
    gates_sorted = gwts.ravel()[perm].astype(np.float32)

    # ---- load balance: core e owns expert e's first <=NT1 slots (segment A);
    # overflow beyond NT1 is cut into NT2-sized pieces placed in other cores'
    # segment B (with the donor expert's weights as the second weight set).
    nt2 = NT2
    while True:
        pieces = []
        for e in range(E):
            for s0 in range(NT1, int(counts[e]), nt2):
                pieces.append((e, cum[e] + s0, min(nt2, int(counts[e]) - s0)))
        if len(pieces) <= N_CORES:
            break
        nt2 += 128
    NT = NT1 + nt2
    NTG = NT // 128
    chunks, groups, _na = _chunk_plan(nt2)
    offs = [sum(chunks[:i]) for i in range(len(chunks) + 1)]
    group_bounds = [(offs[ci[0]], offs[ci[-1] + 1]) for ci in groups]

    io_dtype = ml_dtypes.bfloat16

    def permute_x(xt):
        # xt [D, NT] -> [128, sum_g KD*gs]: per partition, per group,
        # (kc, token) contiguous
        xr = xt.reshape(KD, 128, NT)
        parts = [
            xr[:, :, g0:g1].transpose(1, 0, 2).reshape(128, -1)
            for (g0, g1) in group_bounds
        ]
        return np.ascontiguousarray(np.concatenate(parts, axis=1))

    def w1p(e):
        # [D, F] -> [128, FB, KD, 128] flattened: fb-major weight blocks
        return np.ascontiguousarray(
            w1[e].astype(io_dtype).reshape(KD, 128, FB, 128)
            .transpose(1, 2, 0, 3).reshape(128, FB * KD * 128))

    def w2p(e):
        return np.ascontiguousarray(
            w2[e].astype(io_dtype).reshape(FB, 128, D)
            .transpose(1, 0, 2).reshape(128, FB * D))

    def b1c(e):
        return np.ascontiguousarray(b1[e].reshape(FB, 128).T)

    in_maps = []
    core_sel = []   # (sorted-slot indices, device rows) per core
    for c in range(E):
        nA = min(int(counts[c]), NT1)
        slots_a = np.arange(cum[c], cum[c] + nA)
        if c < len(pieces):
            eB, sB0, nB = pieces[c]
            slots_b = np.arange(sB0, sB0 + nB)
        else:
            eB, slots_b = c, np.arange(0)
        sel = np.concatenate([slots_a, slots_b])
        rows = np.concatenate([np.arange(nA), NT1 + np.arange(len(slots_b))])
        core_sel.append((sel, rows))

        xt = np.zeros((D, NT), io_dtype)
        xt[:, rows] = xf[slot_tok[sel]].astype(io_dtype).T
        gate = np.zeros(NT, np.float32)
        gate[rows] = gates_sorted[sel]
        in_maps.append({
            "xp": permute_x(xt),
            "w1a": w1p(c), "w1b": w1p(eB),
            "w2a": w2p(c), "w2b": w2p(eB),
            "aux": np.ascontiguousarray(np.concatenate(
                [b1c(c), b1c(eB), gate.reshape(NTG, 128).T], axis=1)),
        })

    def run_device():
        if nt2 not in _PROGRAM_CACHE:
            _PROGRAM_CACHE[nt2] = _build_program(nt2)
        nc = _PROGRAM_CACHE[nt2]
        res = run_bass_kernel_spmd(nc, in_maps, list(range(N_CORES)), trace=TRACE)
        if TRACE and res.exec_time_ns is not None:
            print(f"HW exec time: {res.exec_time_ns} ns")
        return [res.results[c]["y"] for c in range(N_CORES)]

    out_slots = np.zeros((T * TOP_K, D), np.float32)
    try:
        try:
            y_cores = run_device()
        except Exception:
            # transient device errors (e.g. NRT exec-unit unrecoverable)
            # are usually gone on retry with a freshly built program
            _PROGRAM_CACHE.clear()
            y_cores = run_device()
        for c in range(N_CORES):
            sel, rows = core_sel[c]
            out_slots[perm[sel]] = y_cores[c].astype(np.float32)[rows]
    except Exception as exc:
        # last resort: identical math on the host so the result is still
        # correct even if the accelerator path is down
        import sys
        print(f"device path failed twice ({exc!r}); computing FFN on host",
              file=sys.stderr)
        for e in range(E):
            n = int(counts[e])
            toks = slot_tok[cum[e]:cum[e] + n]
            h = np.maximum(xf[toks] @ w1[e] + b1[e], 0.0)
            y = (h @ w2[e]) * gates_sorted[cum[e]:cum[e] + n, None]
            out_slots[perm[cum[e]:cum[e] + n]] = y.astype(np.float32)

    # ---- unshard: sum the K slots per token, add the combine@b2 bias term.
    out = out_slots.reshape(T, TOP_K, D).sum(axis=1)
    combine = np.zeros((T, E), np.float32)
    np.put_along_axis(combine, top_idx, gwts.astype(np.float32), axis=1)
    out += combine @ b2

    return out.reshape(B, S, D).astype(np.float32)
